# revision 1
# baseline (speedup 1.0000x reference)
"""Bahdanau-attention scoring kernel for Trainium2 (8 NeuronCores).

reference computation:
  enc = transpose(encoderOutputs, (1,0,2))            # [B,S,H]
  energy = tanh(concat([hidden bcast, enc]) @ W^T(2H contraction) + b)
  scores = energy . v ; softmax over S -> [B,1,S]

decomposition used here:
  energy[b,s,h] = tanh( enc[b,s,:] @ W2[h,:] + (hidden[b,:] @ W1[h,:] + b[h]) )
  with W1 = W[:, :H], W2 = W[:, H:].
  The hidden term ("ubias") is per-(b,h), computed once on-device, and folded
  into the tanh as the ScalarE activation's per-partition bias.

sharding: data-parallel over batch B=32 -> 4 batches per core.
Per-core kernel layout:
  - energy tiles [h=128 part, rows=512 free] via fp32r matmuls
    (lhsT = W2T k-chunk x h-chunk, rhs = encT k-chunk x row-block)
  - tanh fused with per-partition ubias on ScalarE
  - v-dot: DVE accumulates acc += tanh_chunk * v_chunk per h-chunk, then one
    matmul per row-block with a one-hot ones column reduces partitions and
    lands batch bb's scores on psum partition 32*bb (engine partition bases
    must be 32-aligned)
  - softmax over S on a [128, 2048] sbuf tile (4 used partitions), out f32

toolchain notes (this container):
  - walrus here accepts only ONE sync wait per instruction; _split_multiwaits
    rewrites the BIR to single-wait NoOp chains (hooked via nc.to_json_bytes)
  - fp32r matmuls need fp32r-declared producers; inputs are pre-rounded on
    the host (RNE to the fp32r grid) and declared float32r in DRAM
"""

import json
import sys
import types

import numpy as np

H = 1024
S = 2048
B = 32
NCORES = 8
B_LOC = B // NCORES          # 4 batches per core
R = S * B_LOC                # 8192 rows per core (b-major: r = b*S + s)
NBLK = R // 512              # 16 row blocks of 512
KC = H // 128                # 8 contraction chunks
HC = H // 128                # 8 h chunks


def _install_ntff_hook():
    """Install antenv.axon_hooks shim so trace=True works under axon."""
    if "antenv.axon_hooks" in sys.modules:
        return
    try:
        from trn_agent_boot.trn_boot import _ntff_profile_via_ctypes

        hook = _ntff_profile_via_ctypes("/opt/axon/libaxon_pjrt.so")
    except Exception:
        hook = None
    mod = types.ModuleType("antenv.axon_hooks")
    mod._hook = hook
    mod.get_axon_ntff_profile_hook = lambda: mod._hook

    def _set(h):
        mod._hook = h

    mod.set_axon_ntff_profile_hook = _set
    sys.modules["antenv.axon_hooks"] = mod


def _split_multiwaits(bir):
    """This walrus build supports one sync wait per instruction: split
    longer on_wait lists into single-wait NoOps on the same engine."""
    for fn in bir["functions"]:
        for blk in fn["blocks"]:
            out = []
            for inst in blk["instructions"]:
                si = inst.get("sync_info")
                ow = (si or {}).get("on_wait") or []
                if len(ow) > 1:
                    for j, w in enumerate(ow[:-1]):
                        out.append(
                            {
                                "debug": inst.get("debug", 0),
                                "engine": inst["engine"],
                                "ins": [],
                                "name": f"{inst['name']}_sw{j}",
                                "opcode": "NoOp",
                                "outs": [],
                                "sync_info": {"on_wait": [w], "on_update": []},
                                "text_hint": "waitsplit",
                            }
                        )
                    si["on_wait"] = [ow[-1]]
                out.append(inst)
            blk["instructions"] = out
    return bir


def _patch_json(nc):
    orig = nc.to_json_bytes

    def patched():
        return json.dumps(_split_multiwaits(json.loads(orig()))).encode()

    nc.to_json_bytes = patched


def build_kernel():
    import concourse.bass as bass
    import concourse.tile as tile
    from concourse import mybir
    from concourse.masks import make_identity

    f32 = mybir.dt.float32
    f32r = mybir.dt.float32r
    AF = mybir.ActivationFunctionType

    nc = bass.Bass("TRN2", target_bir_lowering=False, debug=False, num_devices=1)

    # fp32r-consumed inputs are declared float32r; the host pre-rounds their
    # values (fp32r = fp32 with mantissa RNE-rounded, 12 low bits dropped).
    enc_t = nc.dram_tensor("enc_t", [H, R], f32r, kind="ExternalInput").ap()
    h_t = nc.dram_tensor("h_t", [128, KC * B_LOC], f32r, kind="ExternalInput").ap()
    w1t = nc.dram_tensor("w1t", [H, H], f32r, kind="ExternalInput").ap()
    w2t = nc.dram_tensor("w2t", [H, H], f32r, kind="ExternalInput").ap()
    bcol = nc.dram_tensor("bcol", [128, HC], f32, kind="ExternalInput").ap()
    vcol = nc.dram_tensor("vcol", [128, HC], f32, kind="ExternalInput").ap()
    onesoh = nc.dram_tensor("onesoh", [128, B_LOC * 128], f32r, kind="ExternalInput").ap()
    out = nc.dram_tensor("out", [B_LOC, S], f32, kind="ExternalOutput").ap()

    with tile.TileContext(nc) as tc:
        with (
            tc.tile_pool(name="consts", bufs=1) as consts,
            tc.tile_pool(name="w1p", bufs=1) as w1p,
            tc.tile_pool(name="w2p", bufs=1) as w2p,
            tc.tile_pool(name="encp", bufs=2) as encp,
            tc.tile_pool(name="tanp", bufs=3) as tanp,
            tc.tile_pool(name="tmpp", bufs=2) as tmpp,
            tc.tile_pool(name="accp", bufs=2) as accp,
            tc.tile_pool(name="scorep", bufs=1) as scorep,
            tc.tile_pool(name="softp", bufs=1) as softp,
            tc.tile_pool(name="ep0", bufs=4, space="PSUM") as ep0,      # blk0 kc-outer
            tc.tile_pool(name="epsum", bufs=2, space="PSUM") as epsum,  # blks >= 1
            tc.tile_pool(name="spsum", bufs=2, space="PSUM") as spsum,  # scores + ubias
        ):
            # ---- W2T lower halves + enc block 0 first on the SP queue -----
            # (the first matmuls need w2sb[:, kc, 0:512] + et0[kc]; everything
            # small rides the gpsimd queue in parallel)
            w2sb = w2p.tile([128, KC, H], f32r, tag="w2sb")
            et0 = []
            for kc in range(KC):
                nc.sync.dma_start(
                    w2sb[:, kc, 0:512], w2t[kc * 128 : (kc + 1) * 128, 0:512]
                )
                et = encp.tile([128, 512], f32r, tag=f"enc{kc}")
                nc.sync.dma_start(et[:], enc_t[kc * 128 : (kc + 1) * 128, 0:512])
                et0.append(et)
            for kc in range(KC):
                nc.sync.dma_start(
                    w2sb[:, kc, 512:H], w2t[kc * 128 : (kc + 1) * 128, 512:H]
                )

            # ---- small constants on the gpsimd queue ----------------------
            h_sb = consts.tile([128, KC, B_LOC], f32r, tag="h_sb")
            nc.gpsimd.dma_start(h_sb[:], h_t.rearrange("p (c b) -> p c b", c=KC))
            bcol_sb = consts.tile([128, HC], f32, tag="bcol_sb")
            nc.gpsimd.dma_start(bcol_sb[:], bcol[:])
            vcol_sb = consts.tile([128, HC], f32, tag="vcol_sb")
            nc.gpsimd.dma_start(vcol_sb[:], vcol[:])

            # ones one-hot for the partition-sum matmul: column 32*bb is 1
            ones_oh = consts.tile([128, B_LOC, 128], f32r, tag="ones_oh")
            nc.gpsimd.dma_start(
                ones_oh[:], onesoh.rearrange("p (b m) -> p b m", b=B_LOC)
            )

            # ---- W1T resident like W2T, on the ACT HWDGE queue ------------
            w1sb = w1p.tile([128, KC, H], f32r, tag="w1sb")
            for kc in range(KC):
                nc.scalar.dma_start(w1sb[:, kc, :], w1t[kc * 128 : (kc + 1) * 128, :])

            # 4x4 identity for the tiny PE transposes of uT
            idt = consts.tile([B_LOC, B_LOC], f32, tag="idt")
            make_identity(nc, idt[:])

            # uT[b, h] = (hidden @ W1^T)[b, h] via wide-N matmuls with the
            # 4-column hidden as the stationary operand (cheap weight loads),
            # then 8 tiny PE transposes to get ubias in [h-part, b] layout
            uts = consts.tile([B_LOC, H], f32, tag="uts")
            ubias = consts.tile([128, HC, B_LOC], f32, tag="ubias")

            def emit_u_half(nh):
                upt = spsum.tile([128, 512], f32, tag="sp")
                for kc in range(KC):
                    nc.tensor.matmul(
                        upt[0:B_LOC, :],
                        h_sb[:, kc, :],
                        w1sb[:, kc, nh * 512 : (nh + 1) * 512],
                        start=(kc == 0),
                        stop=(kc == KC - 1),
                        skip_group_check=True,
                    )
                nc.vector.tensor_copy(
                    uts[0:B_LOC, nh * 512 : (nh + 1) * 512], upt[0:B_LOC, :]
                )
                for hc in range(nh * 4, nh * 4 + 4):
                    trp = spsum.tile([128, 512], f32, tag="sp")
                    nc.tensor.transpose(
                        trp[:, 0:B_LOC],
                        uts[0:B_LOC, hc * 128 : (hc + 1) * 128],
                        idt[:],
                    )
                    nc.vector.tensor_scalar_add(
                        ubias[:, hc, :], trp[:, 0:B_LOC], bcol_sb[:, hc : hc + 1]
                    )

            # ---- main loop over 16 row blocks -----------------------------
            # batch bb's scores live on partition 32*bb
            scores = scorep.tile([128, S], f32, tag="scores")
            nc.vector.memset(scores[:], 0.0)

            pending_sum = None  # (acc tile, bb, sb) awaiting partition-sum MM

            def emit_sum(pending):
                acc, bb, sb = pending
                mw = 32 * bb + 1
                sp = spsum.tile([128, 512], f32, tag="sp")
                nc.tensor.matmul(
                    sp[0:mw, :],
                    ones_oh[:, bb, 0:mw],
                    acc[:],
                    start=True,
                    stop=True,
                    skip_group_check=True,
                )
                nc.vector.tensor_copy(
                    scores[32 * bb : 32 * bb + 1, sb * 512 : (sb + 1) * 512],
                    sp[32 * bb : 32 * bb + 1, :],
                )

            for blk in range(NBLK):
                bb = blk // (S // 512)       # batch of this block
                sb = blk % (S // 512)        # block index within the batch
                if blk == 0:
                    etiles = et0
                else:
                    etiles = []
                    for kc in range(KC):
                        et = encp.tile([128, 512], f32r, tag=f"enc{kc}")
                        eng = nc.sync
                        eng.dma_start(
                            et[:],
                            enc_t[
                                kc * 128 : (kc + 1) * 128, blk * 512 : (blk + 1) * 512
                            ],
                        )
                        etiles.append(et)

                acc = accp.tile([128, 512], f32r, tag="acc")

                def postproc(ep, hc):
                    # tanh with fused ubias, then DVE v-scale + accumulate
                    tt = tanp.tile([128, 512], f32, tag="tt")
                    nc.scalar.activation(
                        tt[:], ep[:], AF.Tanh,
                        bias=ubias[:, hc, bb : bb + 1], scale=1.0,
                    )
                    if hc == 0:
                        nc.vector.tensor_scalar_mul(
                            acc[:], tt[:], vcol_sb[:, hc : hc + 1]
                        )
                    else:
                        tmp = tmpp.tile([128, 512], f32, tag="tmp")
                        nc.vector.tensor_scalar_mul(
                            tmp[:], tt[:], vcol_sb[:, hc : hc + 1]
                        )
                        nc.vector.tensor_tensor(
                            acc[:], tmp[:], acc[:], op=mybir.AluOpType.add
                        )

                if blk == 0:
                    # kc-outer halves: PE can start as soon as the first
                    # (w2, enc) pair lands instead of waiting for all 8
                    for half in range(2):
                        hcs = range(half * 4, half * 4 + 4)
                        eps = {}
                        for hc in hcs:
                            e0t = ep0.tile([128, 512], f32, tag="e0")
                            eps[hc] = e0t
                        for kc in range(KC):
                            for hc in hcs:
                                nc.tensor.matmul(
                                    eps[hc][:],
                                    w2sb[:, kc, hc * 128 : (hc + 1) * 128],
                                    etiles[kc][:],
                                    start=(kc == 0),
                                    stop=(kc == KC - 1),
                                    skip_group_check=True,
                                )
                        emit_u_half(half)
                        for hc in hcs:
                            postproc(eps[hc], hc)
                else:
                    for hc in range(HC):
                        ep = epsum.tile([128, 512], f32, tag="ep")
                        last_chunk = blk == NBLK - 1 and hc == HC - 1
                        if last_chunk:
                            # split the very last energy group into two
                            # 256-column halves so the tanh/v-dot chain (and
                            # with it the final partition-sum) starts half a
                            # group earlier - trims the end-of-stream stall
                            tt_l = tanp.tile([128, 512], f32, tag="tt")
                            tmp_l = tmpp.tile([128, 512], f32, tag="tmp")
                            for half in range(2):
                                sl = slice(half * 256, half * 256 + 256)
                                for kc in range(KC):
                                    nc.tensor.matmul(
                                        ep[:, sl],
                                        w2sb[:, kc, hc * 128 : (hc + 1) * 128],
                                        etiles[kc][:, sl],
                                        start=(kc == 0),
                                        stop=(kc == KC - 1),
                                        skip_group_check=True,
                                    )
                                nc.scalar.activation(
                                    tt_l[:, sl], ep[:, sl], AF.Tanh,
                                    bias=ubias[:, hc, bb : bb + 1], scale=1.0,
                                )
                                nc.vector.tensor_scalar_mul(
                                    tmp_l[:, sl], tt_l[:, sl],
                                    vcol_sb[:, hc : hc + 1],
                                )
                                nc.vector.tensor_tensor(
                                    acc[:, sl], tmp_l[:, sl], acc[:, sl],
                                    op=mybir.AluOpType.add,
                                )
                            continue
                        for kc in range(KC):
                            nc.tensor.matmul(
                                ep[:],
                                w2sb[:, kc, hc * 128 : (hc + 1) * 128],
                                etiles[kc][:],
                                start=(kc == 0),
                                stop=(kc == KC - 1),
                            )
                        if hc == 0 and pending_sum is not None:
                            emit_sum(pending_sum)
                            pending_sum = None
                        postproc(ep, hc)

                pending_sum = (acc, bb, sb)

            emit_sum(pending_sum)

            # ---- softmax over S per batch ---------------------------------
            # no max-subtraction: |scores| <= ||v||_1 (~25), exp() is safely
            # inside fp32 range, and softmax is shift-invariant
            esc = softp.tile([128, S], f32, tag="esc")
            ssum = softp.tile([128, 1], f32, tag="ssum")
            nc.scalar.activation(
                esc[:], scores[:], AF.Exp, bias=0.0, scale=1.0,
                accum_out=ssum[:],
            )
            rsum = softp.tile([128, 1], f32, tag="rsum")
            nc.vector.reciprocal(rsum[:], ssum[:])
            prob = softp.tile([128, S], f32, tag="prob")
            nc.vector.tensor_scalar_mul(prob[:], esc[:], rsum[:])
            for bb in range(B_LOC):
                nc.sync.dma_start(
                    out[bb : bb + 1, :], prob[32 * bb : 32 * bb + 1, :]
                )

    _patch_json(nc)
    return nc


_NC_CACHE = None


def _get_nc():
    global _NC_CACHE
    if _NC_CACHE is None:
        _NC_CACHE = build_kernel()
    return _NC_CACHE


def round_fp32r(x):
    """RNE-round fp32 values to the fp32r grid (drop low 12 mantissa bits)."""
    u = np.ascontiguousarray(x, dtype=np.float32).view(np.uint32)
    r = (u + 0x7FF + ((u >> 12) & 1)) & 0xFFFFF000
    return r.astype(np.uint32).view(np.float32)


def shard_inputs(hidden, encoderOutputs, W, b, v):
    """Host-side prep: per-core input dict list."""
    hidden = np.ascontiguousarray(hidden, dtype=np.float32)
    W = np.ascontiguousarray(W, dtype=np.float32)
    b = np.ascontiguousarray(b, dtype=np.float32)
    v = np.ascontiguousarray(v, dtype=np.float32)

    w1t = round_fp32r(np.ascontiguousarray(W[:, :H].T))  # [k, h]
    w2t = round_fp32r(np.ascontiguousarray(W[:, H:].T))  # [k, h]
    bcol = np.ascontiguousarray(b.reshape(HC, 128).T)    # [128, hc]
    vcol = np.ascontiguousarray(v.reshape(HC, 128).T)   # [128, hc]
    onesoh = np.zeros((128, B_LOC, 128), np.float32)
    for bb in range(B_LOC):
        onesoh[:, bb, 32 * bb] = 1.0
    onesoh = np.ascontiguousarray(onesoh.reshape(128, B_LOC * 128))

    # [H, B, S] single big transpose, then per-core contiguous slices
    encT = round_fp32r(
        np.transpose(np.asarray(encoderOutputs, dtype=np.float32), (2, 1, 0))
    )

    in_maps = []
    for i in range(NCORES):
        b0 = i * B_LOC
        enc_c = np.ascontiguousarray(encT[:, b0 : b0 + B_LOC, :]).reshape(H, R)
        hid_c = hidden[b0 : b0 + B_LOC]                  # [4, H]
        h_t = round_fp32r(
            hid_c.T.reshape(KC, 128, B_LOC).transpose(1, 0, 2).reshape(128, KC * B_LOC)
        )
        in_maps.append(
            {
                "enc_t": enc_c,
                "h_t": h_t,
                "w1t": w1t,
                "w2t": w2t,
                "bcol": bcol,
                "vcol": vcol,
                "onesoh": onesoh,
            }
        )
    return in_maps


def run(in_maps, trace=False):
    if trace:
        _install_ntff_hook()
    from concourse import bass_utils

    nc = _get_nc()
    res = bass_utils.run_bass_kernel_spmd(
        nc, in_maps, core_ids=list(range(NCORES)), trace=trace
    )
    return res


def kernel(hidden, encoderOutputs, W, b, v):
    in_maps = shard_inputs(hidden, encoderOutputs, W, b, v)
    res = run(in_maps, trace=False)
    outs = [res.results[i]["out"] for i in range(NCORES)]   # each [4, S]
    full = np.concatenate(outs, axis=0)                     # [32, S]
    return full[:, None, :].astype(np.float32)              # [32, 1, S]



# revision 3
# speedup vs baseline: 1.1027x; 1.1027x over previous
"""Bahdanau-attention scoring kernel for Trainium2 (8 NeuronCores).

reference computation:
  enc = transpose(encoderOutputs, (1,0,2))            # [B,S,H]
  energy = tanh(concat([hidden bcast, enc]) @ W^T(2H contraction) + b)
  scores = energy . v ; softmax over S -> [B,1,S]

decomposition used here:
  energy[b,s,h] = tanh( enc[b,s,:] @ W2[h,:] + ubias[b,h] )
  with W1 = W[:, :H], W2 = W[:, H:], ubias = hidden @ W1^T + b (host-side:
  it is a [B,H] mat-vec scale problem, negligible vs the [B,S,H] energy).

sharding: data-parallel over batch B=32 -> 4 batches per core.
Per-core kernel layout:
  - energy tiles [h=128 part, rows=512 free] via bf16 matmuls
    (lhsT = W2T k-chunk x h-chunk, rhs = encT k-chunk x row-block), fp32 PSUM
  - tanh fused with per-partition ubias on ScalarE, bf16 out
  - v-dot: DVE accumulates acc += tanh_chunk * v_chunk per h-chunk (bf16,
    2x DVE mode), then one single-column ones matmul per row-block reduces
    partitions, accumulating batch bb's scores directly into PSUM score
    bank sb at partition 32*bb (tile_position=(0, 32*bb))
  - softmax over S read straight from the 4 score banks: exp+accum per bank
    (emitted as soon as the bank's last writer lands), combine sums, scale

toolchain notes (this container):
  - walrus here accepts only ONE sync wait per instruction; _split_multiwaits
    rewrites the BIR to single-wait NoOp chains (hooked via nc.to_json_bytes)
"""

import json
import sys
import types

import numpy as np

H = 1024
S = 2048
B = 32
NCORES = 8
B_LOC = B // NCORES          # 4 batches per core
R = S * B_LOC                # 8192 rows per core (b-major: r = b*S + s)
NBLK = R // 512              # 16 row blocks of 512
KC = H // 128                # 8 contraction chunks
HC = H // 128                # 8 h chunks


def _install_ntff_hook():
    """Install antenv.axon_hooks shim so trace=True works under axon."""
    if "antenv.axon_hooks" in sys.modules:
        return
    try:
        from trn_agent_boot.trn_boot import _ntff_profile_via_ctypes

        hook = _ntff_profile_via_ctypes("/opt/axon/libaxon_pjrt.so")
    except Exception:
        hook = None
    mod = types.ModuleType("antenv.axon_hooks")
    mod._hook = hook
    mod.get_axon_ntff_profile_hook = lambda: mod._hook

    def _set(h):
        mod._hook = h

    mod.set_axon_ntff_profile_hook = _set
    sys.modules["antenv.axon_hooks"] = mod


def _split_multiwaits(bir):
    """This walrus build supports one sync wait per instruction: split
    longer on_wait lists into single-wait NoOps on the same engine."""
    for fn in bir["functions"]:
        for blk in fn["blocks"]:
            out = []
            for inst in blk["instructions"]:
                si = inst.get("sync_info")
                ow = (si or {}).get("on_wait") or []
                if len(ow) > 1:
                    for j, w in enumerate(ow[:-1]):
                        out.append(
                            {
                                "debug": inst.get("debug", 0),
                                "engine": inst["engine"],
                                "ins": [],
                                "name": f"{inst['name']}_sw{j}",
                                "opcode": "NoOp",
                                "outs": [],
                                "sync_info": {"on_wait": [w], "on_update": []},
                                "text_hint": "waitsplit",
                            }
                        )
                    si["on_wait"] = [ow[-1]]
                out.append(inst)
            blk["instructions"] = out
    return bir


def _patch_json(nc):
    orig = nc.to_json_bytes

    def patched():
        return json.dumps(_split_multiwaits(json.loads(orig()))).encode()

    nc.to_json_bytes = patched


def build_kernel():
    import concourse.bass as bass
    import concourse.tile as tile
    from concourse import mybir

    f32 = mybir.dt.float32
    bf16 = mybir.dt.bfloat16
    AF = mybir.ActivationFunctionType

    nc = bass.Bass("TRN2", target_bir_lowering=False, debug=False, num_devices=1)

    enc_t = nc.dram_tensor("enc_t", [H, R], bf16, kind="ExternalInput").ap()
    w2t = nc.dram_tensor("w2t", [H, H], bf16, kind="ExternalInput").ap()
    ubias = nc.dram_tensor("ubias", [128, HC * B_LOC], f32, kind="ExternalInput").ap()
    vcol = nc.dram_tensor("vcol", [128, HC], f32, kind="ExternalInput").ap()
    out = nc.dram_tensor("out", [B_LOC, S], f32, kind="ExternalOutput").ap()

    with tile.TileContext(nc) as tc:
        with (
            tc.tile_pool(name="consts", bufs=1) as consts,
            tc.tile_pool(name="w2p", bufs=1) as w2p,
            tc.tile_pool(name="encp", bufs=3) as encp,
            tc.tile_pool(name="tanp", bufs=3) as tanp,
            tc.tile_pool(name="tmpp", bufs=2) as tmpp,
            tc.tile_pool(name="accp", bufs=2) as accp,
            tc.tile_pool(name="softp", bufs=1) as softp,
            tc.tile_pool(name="ep", bufs=4, space="PSUM") as epp,      # energy
            tc.tile_pool(name="scorep", bufs=1, space="PSUM") as scorep,  # 4 banks
        ):
            # ---- W2T lower halves + enc block 0 first on the SP queue -----
            # (the first matmuls need w2sb[:, kc, 0:512] + et0[kc])
            w2sb = w2p.tile([128, KC, H], bf16, tag="w2sb")
            et0 = []
            for kc in range(KC):
                nc.sync.dma_start(
                    w2sb[:, kc, 0:512], w2t[kc * 128 : (kc + 1) * 128, 0:512]
                )
                et = encp.tile([128, 512], bf16, tag=f"enc{kc}")
                nc.sync.dma_start(et[:], enc_t[kc * 128 : (kc + 1) * 128, 0:512])
                et0.append(et)
            for kc in range(KC):
                nc.sync.dma_start(
                    w2sb[:, kc, 512:H], w2t[kc * 128 : (kc + 1) * 128, 512:H]
                )

            # ---- small constants on the gpsimd queue ----------------------
            ub_sb = consts.tile([128, HC, B_LOC], f32, tag="ub_sb")
            nc.gpsimd.dma_start(ub_sb[:], ubias.rearrange("p (c b) -> p c b", c=HC))
            vcol_sb = consts.tile([128, HC], f32, tag="vcol_sb")
            nc.gpsimd.dma_start(vcol_sb[:], vcol[:])

            # single ones column for the partition-sum matmuls
            ones1 = consts.tile([128, 1], bf16, tag="ones1")
            nc.vector.memset(ones1[:], 1.0)

            # ---- 4 persistent PSUM score banks; zero the unused partitions
            # so the later exp() never sees stale garbage -------------------
            sc_banks = []
            for sb in range(S // 512):
                scb = scorep.tile([128, 512], f32, tag=f"sc{sb}")
                nc.vector.memset(scb[:], 0.0)
                sc_banks.append(scb)

            esc = softp.tile([128, S], f32, tag="esc")
            ssums = consts.tile([128, S // 512], f32, tag="ssums")

            # ---- main loop over 16 row blocks -----------------------------
            # batch bb's scores live on partition 32*bb of score bank sb
            pending_sum = None  # (acc tile, bb, sb) awaiting partition-sum MM

            def emit_sum(pending):
                acc, bb, sb = pending
                nc.tensor.matmul(
                    sc_banks[sb][32 * bb : 32 * bb + 1, :],
                    ones1[:, 0:1],
                    acc[:],
                    start=True,
                    stop=True,
                    skip_group_check=True,
                    tile_position=(0, 32 * bb),
                )
                if bb == B_LOC - 1:
                    # bank sb complete: exp + per-partition sums, overlapped
                    # with the remaining blocks' compute
                    nc.scalar.activation(
                        esc[:, sb * 512 : (sb + 1) * 512],
                        sc_banks[sb][:],
                        AF.Exp,
                        bias=0.0,
                        scale=1.0,
                        accum_out=ssums[:, sb : sb + 1],
                    )

            for blk in range(NBLK):
                bb = blk // (S // 512)       # batch of this block
                sb = blk % (S // 512)        # block index within the batch
                if blk == 0:
                    etiles = et0
                else:
                    etiles = []
                    for kc in range(KC):
                        et = encp.tile([128, 512], bf16, tag=f"enc{kc}")
                        nc.sync.dma_start(
                            et[:],
                            enc_t[
                                kc * 128 : (kc + 1) * 128, blk * 512 : (blk + 1) * 512
                            ],
                        )
                        etiles.append(et)

                acc = accp.tile([128, 512], bf16, tag="acc")

                def postproc(ep, hc):
                    # tanh with fused ubias, then DVE v-scale + accumulate
                    tt = tanp.tile([128, 512], bf16, tag="tt")
                    nc.scalar.activation(
                        tt[:], ep[:], AF.Tanh,
                        bias=ub_sb[:, hc, bb : bb + 1], scale=1.0,
                    )
                    if hc == 0:
                        nc.vector.tensor_scalar_mul(
                            acc[:], tt[:], vcol_sb[:, hc : hc + 1]
                        )
                    else:
                        tmp = tmpp.tile([128, 512], bf16, tag="tmp")
                        nc.vector.tensor_scalar_mul(
                            tmp[:], tt[:], vcol_sb[:, hc : hc + 1]
                        )
                        nc.vector.tensor_tensor(
                            acc[:], tmp[:], acc[:], op=mybir.AluOpType.add
                        )

                if blk == 0:
                    # kc-outer halves: PE can start as soon as the first
                    # (w2, enc) pair lands instead of waiting for all 8
                    for half in range(2):
                        hcs = range(half * 4, half * 4 + 4)
                        eps = {}
                        for hc in hcs:
                            e0t = epp.tile([128, 512], f32, tag="ep")
                            eps[hc] = e0t
                        for kc in range(KC):
                            for hc in hcs:
                                nc.tensor.matmul(
                                    eps[hc][:],
                                    w2sb[:, kc, hc * 128 : (hc + 1) * 128],
                                    etiles[kc][:],
                                    start=(kc == 0),
                                    stop=(kc == KC - 1),
                                    skip_group_check=True,
                                )
                        for hc in hcs:
                            postproc(eps[hc], hc)
                else:
                    for hc in range(HC):
                        ep = epp.tile([128, 512], f32, tag="ep")
                        last_chunk = blk == NBLK - 1 and hc == HC - 1
                        if last_chunk:
                            # split the very last energy group into two
                            # 256-column halves so the tanh/v-dot chain (and
                            # with it the final partition-sum) starts half a
                            # group earlier - trims the end-of-stream stall
                            tt_l = tanp.tile([128, 512], bf16, tag="tt")
                            tmp_l = tmpp.tile([128, 512], bf16, tag="tmp")
                            for half in range(2):
                                sl = slice(half * 256, half * 256 + 256)
                                for kc in range(KC):
                                    nc.tensor.matmul(
                                        ep[:, sl],
                                        w2sb[:, kc, hc * 128 : (hc + 1) * 128],
                                        etiles[kc][:, sl],
                                        start=(kc == 0),
                                        stop=(kc == KC - 1),
                                        skip_group_check=True,
                                    )
                                nc.scalar.activation(
                                    tt_l[:, sl], ep[:, sl], AF.Tanh,
                                    bias=ub_sb[:, hc, bb : bb + 1], scale=1.0,
                                )
                                nc.vector.tensor_scalar_mul(
                                    tmp_l[:, sl], tt_l[:, sl],
                                    vcol_sb[:, hc : hc + 1],
                                )
                                nc.vector.tensor_tensor(
                                    acc[:, sl], tmp_l[:, sl], acc[:, sl],
                                    op=mybir.AluOpType.add,
                                )
                            continue
                        for kc in range(KC):
                            nc.tensor.matmul(
                                ep[:],
                                w2sb[:, kc, hc * 128 : (hc + 1) * 128],
                                etiles[kc][:],
                                start=(kc == 0),
                                stop=(kc == KC - 1),
                            )
                        if hc == 0 and pending_sum is not None:
                            emit_sum(pending_sum)
                            pending_sum = None
                        postproc(ep, hc)

                pending_sum = (acc, bb, sb)

            emit_sum(pending_sum)

            # ---- softmax over S per batch ---------------------------------
            # no max-subtraction: |scores| <= ||v||_1 (~25), exp() is safely
            # inside fp32 range, and softmax is shift-invariant
            ssum = softp.tile([128, 1], f32, tag="ssum")
            nc.vector.tensor_tensor(
                ssum[:], ssums[:, 0:1], ssums[:, 1:2], op=mybir.AluOpType.add
            )
            ssum2 = softp.tile([128, 1], f32, tag="ssum2")
            nc.vector.tensor_tensor(
                ssum2[:], ssums[:, 2:3], ssums[:, 3:4], op=mybir.AluOpType.add
            )
            nc.vector.tensor_tensor(
                ssum[:], ssum[:], ssum2[:], op=mybir.AluOpType.add
            )
            rsum = softp.tile([128, 1], f32, tag="rsum")
            nc.vector.reciprocal(rsum[:], ssum[:])
            prob = softp.tile([128, S], f32, tag="prob")
            nc.vector.tensor_scalar_mul(prob[:], esc[:], rsum[:])
            for bb in range(B_LOC):
                nc.sync.dma_start(
                    out[bb : bb + 1, :], prob[32 * bb : 32 * bb + 1, :]
                )

    _patch_json(nc)
    return nc


_NC_CACHE = None


def _get_nc():
    global _NC_CACHE
    if _NC_CACHE is None:
        _NC_CACHE = build_kernel()
    return _NC_CACHE


def shard_inputs(hidden, encoderOutputs, W, b, v):
    """Host-side prep: per-core input dict list."""
    import ml_dtypes

    bf16 = ml_dtypes.bfloat16

    hidden = np.ascontiguousarray(hidden, dtype=np.float32)
    W = np.ascontiguousarray(W, dtype=np.float32)
    b = np.ascontiguousarray(b, dtype=np.float32)
    v = np.ascontiguousarray(v, dtype=np.float32)

    w2t = np.ascontiguousarray(W[:, H:].T).astype(bf16)   # [k, h]
    vcol = np.ascontiguousarray(v.reshape(HC, 128).T)     # [128, hc]

    # ubias[b, h] = hidden @ W1^T + b  (tiny [B,H] problem: host fp32)
    ub = hidden @ W[:, :H].T + b[None, :]                 # [B, H]

    # [H, B, S] single big transpose, then per-core contiguous slices
    encT = np.transpose(
        np.asarray(encoderOutputs, dtype=np.float32), (2, 1, 0)
    ).astype(bf16)

    in_maps = []
    for i in range(NCORES):
        b0 = i * B_LOC
        enc_c = np.ascontiguousarray(encT[:, b0 : b0 + B_LOC, :]).reshape(H, R)
        # ubias in [h-part, hc, bb] layout -> [128, HC * B_LOC]
        ub_c = np.ascontiguousarray(
            ub[b0 : b0 + B_LOC].T.reshape(HC, 128, B_LOC).transpose(1, 0, 2)
        ).reshape(128, HC * B_LOC)
        in_maps.append(
            {
                "enc_t": enc_c,
                "w2t": w2t,
                "ubias": ub_c,
                "vcol": vcol,
            }
        )
    return in_maps


def run(in_maps, trace=False):
    if trace:
        _install_ntff_hook()
    from concourse import bass_utils

    nc = _get_nc()
    res = bass_utils.run_bass_kernel_spmd(
        nc, in_maps, core_ids=list(range(NCORES)), trace=trace
    )
    return res


def kernel(hidden, encoderOutputs, W, b, v):
    in_maps = shard_inputs(hidden, encoderOutputs, W, b, v)
    res = run(in_maps, trace=False)
    outs = [res.results[i]["out"] for i in range(NCORES)]   # each [4, S]
    full = np.concatenate(outs, axis=0)                     # [32, S]
    return full[:, None, :].astype(np.float32)              # [32, 1, S]


# revision 9
# speedup vs baseline: 1.3242x; 1.2008x over previous
"""Bahdanau-attention scoring kernel for Trainium2 (8 NeuronCores).

reference computation:
  enc = transpose(encoderOutputs, (1,0,2))            # [B,S,H]
  energy = tanh(concat([hidden bcast, enc]) @ W^T(2H contraction) + b)
  scores = energy . v ; softmax over S -> [B,1,S]

decomposition used here:
  energy[b,s,h] = tanh( enc[b,s,:] @ W2[h,:] + ubias[b,h] )
  with W1 = W[:, :H], W2 = W[:, H:], ubias = hidden @ W1^T + b (host-side:
  it is a [B,H] mat-vec scale problem, negligible vs the [B,S,H] energy).

sharding: data-parallel over batch B=32 -> 4 batches per core.
Per-core kernel layout:
  - energy tiles [h=128 part, rows=512 free] via bf16 matmuls
    (lhsT = W2T k-chunk x h-chunk, rhs = encT k-chunk x row-block), fp32 PSUM
  - tanh fused with per-partition ubias on ScalarE, bf16 out
  - v-dot: DVE accumulates acc += tanh_chunk * v_chunk per h-chunk (bf16,
    2x DVE mode), then one single-column ones matmul per row-block reduces
    partitions, accumulating batch bb's scores directly into PSUM score
    bank sb at partition 32*bb (tile_position=(0, 32*bb))
  - softmax over S read straight from the 4 score banks: exp+accum per bank
    (emitted as soon as the bank's last writer lands), combine sums, scale

toolchain notes (this container):
  - walrus here accepts only ONE sync wait per instruction; _split_multiwaits
    rewrites the BIR to single-wait NoOp chains (hooked via nc.to_json_bytes)
"""

import json
import sys
import types

import numpy as np

H = 1024
S = 2048
B = 32
NCORES = 8
B_LOC = B // NCORES          # 4 batches per core
R = S * B_LOC                # 8192 rows per core (b-major: r = b*S + s)
NBLK = R // 512              # 16 row blocks of 512
KC = H // 128                # 8 contraction chunks
HC = H // 128                # 8 h chunks


def _install_ntff_hook():
    """Install antenv.axon_hooks shim so trace=True works under axon."""
    if "antenv.axon_hooks" in sys.modules:
        return
    try:
        from trn_agent_boot.trn_boot import _ntff_profile_via_ctypes

        hook = _ntff_profile_via_ctypes("/opt/axon/libaxon_pjrt.so")
    except Exception:
        hook = None
    mod = types.ModuleType("antenv.axon_hooks")
    mod._hook = hook
    mod.get_axon_ntff_profile_hook = lambda: mod._hook

    def _set(h):
        mod._hook = h

    mod.set_axon_ntff_profile_hook = _set
    sys.modules["antenv.axon_hooks"] = mod


def _split_multiwaits(bir):
    """This walrus build supports one sync wait per instruction: split
    longer on_wait lists into single-wait NoOps on the same engine."""
    for fn in bir["functions"]:
        for blk in fn["blocks"]:
            out = []
            for inst in blk["instructions"]:
                si = inst.get("sync_info")
                ow = (si or {}).get("on_wait") or []
                if len(ow) > 1:
                    for j, w in enumerate(ow[:-1]):
                        out.append(
                            {
                                "debug": inst.get("debug", 0),
                                "engine": inst["engine"],
                                "ins": [],
                                "name": f"{inst['name']}_sw{j}",
                                "opcode": "NoOp",
                                "outs": [],
                                "sync_info": {"on_wait": [w], "on_update": []},
                                "text_hint": "waitsplit",
                            }
                        )
                    si["on_wait"] = [ow[-1]]
                out.append(inst)
            blk["instructions"] = out
    return bir


def _patch_json(nc):
    orig = nc.to_json_bytes

    def patched():
        return json.dumps(_split_multiwaits(json.loads(orig()))).encode()

    nc.to_json_bytes = patched


def build_kernel():
    import concourse.bass as bass
    import concourse.tile as tile
    from concourse import mybir

    f32 = mybir.dt.float32
    bf16 = mybir.dt.bfloat16
    AF = mybir.ActivationFunctionType

    nc = bass.Bass("TRN2", target_bir_lowering=False, debug=False, num_devices=1)

    enc_t = nc.dram_tensor("enc_t", [H, R], bf16, kind="ExternalInput").ap()
    w2t = nc.dram_tensor("w2t", [H, H], bf16, kind="ExternalInput").ap()
    ubias = nc.dram_tensor("ubias", [128, HC * B_LOC], f32, kind="ExternalInput").ap()
    vcol = nc.dram_tensor("vcol", [128, HC], f32, kind="ExternalInput").ap()
    # unnormalized softmax: exp(scores) rows + per-(batch, bank) partial sums;
    # the host does the final normalize (a [B,S] divide - negligible there)
    out_esc = nc.dram_tensor("out_esc", [B_LOC, S], f32, kind="ExternalOutput").ap()
    out_sum = nc.dram_tensor("out_sum", [B_LOC, S // 512], f32, kind="ExternalOutput").ap()
    enc_t3 = enc_t.rearrange("(c p) r -> p c r", c=KC)
    w2t3 = w2t.rearrange("(c p) r -> p c r", c=KC)

    with tile.TileContext(nc) as tc:
        with (
            tc.tile_pool(name="consts", bufs=1) as consts,
            tc.tile_pool(name="w2p", bufs=1) as w2p,
            tc.tile_pool(name="encp", bufs=3) as encp,
            tc.tile_pool(name="tanp", bufs=3) as tanp,
            tc.tile_pool(name="tmpp", bufs=2) as tmpp,
            tc.tile_pool(name="accp", bufs=2) as accp,
            tc.tile_pool(name="softp", bufs=1) as softp,
            tc.tile_pool(name="ep", bufs=4, space="PSUM") as epp,      # energy
            tc.tile_pool(name="scorep", bufs=1, space="PSUM") as scorep,  # 4 banks
        ):
            # ---- W2T lower halves + enc block 0 first on the SP queue -----
            # (the first matmuls need w2sb[:, kc, 0:512] + et0[kc])
            w2sb = w2p.tile([128, KC, H], bf16, tag="w2sb")
            et0 = []
            for kc in range(KC):
                nc.sync.dma_start(
                    w2sb[:, kc, 0:512], w2t[kc * 128 : (kc + 1) * 128, 0:512]
                )
                et = encp.tile([128, 512], bf16, tag=f"enc{kc}")
                nc.sync.dma_start(et[:], enc_t[kc * 128 : (kc + 1) * 128, 0:512])
                et0.append(et)
            nc.sync.dma_start(w2sb[:, :, 512:H], w2t3[:, :, 512:H])

            # ---- small constants on the gpsimd queue ----------------------
            ub_sb = consts.tile([128, HC, B_LOC], f32, tag="ub_sb")
            nc.gpsimd.dma_start(ub_sb[:], ubias.rearrange("p (c b) -> p c b", c=HC))
            vcol_sb = consts.tile([128, HC], f32, tag="vcol_sb")
            nc.gpsimd.dma_start(vcol_sb[:], vcol[:])

            # single ones column for the partition-sum matmuls
            ones1 = consts.tile([128, 1], bf16, tag="ones1")
            nc.vector.memset(ones1[:], 1.0)

            # ---- 4 persistent PSUM score banks; zero the unused partitions
            # so the later exp() never sees stale garbage -------------------
            sc_banks = []
            for sb in range(S // 512):
                scb = scorep.tile([128, 512], f32, tag=f"sc{sb}")
                nc.vector.memset(scb[:], 0.0)
                sc_banks.append(scb)

            esc = softp.tile([128, S], f32, tag="esc")
            ssums = softp.tile([128, S // 512], f32, tag="ssums")

            # ---- main loop over 16 row blocks -----------------------------
            # batch bb's scores live on partition 32*bb of score bank sb
            pending_sum = None  # (acc tile, bb, sb) awaiting partition-sum MM

            def emit_sum(pending):
                acc, bb, sb = pending
                nc.tensor.matmul(
                    sc_banks[sb][32 * bb : 32 * bb + 1, :],
                    ones1[:, 0:1],
                    acc[:],
                    start=True,
                    stop=True,
                    skip_group_check=True,
                    tile_position=(0, 32 * bb),
                )
                if bb == B_LOC - 1:
                    # bank sb complete: exp + per-partition sums + output DMA,
                    # overlapped with the remaining blocks' compute (only the
                    # last bank's chain lands in the kernel tail)
                    sl = slice(sb * 512, (sb + 1) * 512)
                    nc.scalar.activation(
                        esc[:, sl],
                        sc_banks[sb][:],
                        AF.Exp,
                        bias=0.0,
                        scale=1.0,
                        accum_out=ssums[:, sb : sb + 1],
                    )
                    nc.sync.dma_start(out_esc[0:B_LOC, sl], esc[0:128:32, sl])

            for blk in range(NBLK):
                bb = blk // (S // 512)       # batch of this block
                sb = blk % (S // 512)        # block index within the batch
                if blk == 0:
                    etiles = et0
                else:
                    # one 1 MiB transfer per block: fewer DMA sems, max BW
                    et_all = encp.tile([128, KC, 512], bf16, tag="enc")
                    nc.sync.dma_start(
                        et_all[:], enc_t3[:, :, blk * 512 : (blk + 1) * 512]
                    )
                    etiles = [et_all[:, kc, :] for kc in range(KC)]

                acc = accp.tile([128, 512], bf16, tag="acc")

                def postproc(ep, hc):
                    # tanh with fused ubias, then DVE v-scale + accumulate
                    tt = tanp.tile([128, 512], bf16, tag="tt")
                    nc.scalar.activation(
                        tt[:], ep[:], AF.Tanh,
                        bias=ub_sb[:, hc, bb : bb + 1], scale=1.0,
                    )
                    if hc == 0:
                        nc.vector.tensor_scalar_mul(
                            acc[:], tt[:], vcol_sb[:, hc : hc + 1]
                        )
                    else:
                        tmp = tmpp.tile([128, 512], bf16, tag="tmp")
                        nc.vector.tensor_scalar_mul(
                            tmp[:], tt[:], vcol_sb[:, hc : hc + 1]
                        )
                        nc.vector.tensor_tensor(
                            acc[:], tmp[:], acc[:], op=mybir.AluOpType.add
                        )

                if blk == 0:
                    # kc-outer halves: PE can start as soon as the first
                    # (w2, enc) pair lands instead of waiting for all 8
                    for half in range(2):
                        hcs = range(half * 4, half * 4 + 4)
                        eps = {}
                        for hc in hcs:
                            e0t = epp.tile([128, 512], f32, tag="ep")
                            eps[hc] = e0t
                        for kc in range(KC):
                            for hc in hcs:
                                nc.tensor.matmul(
                                    eps[hc][:],
                                    w2sb[:, kc, hc * 128 : (hc + 1) * 128],
                                    etiles[kc][:],
                                    start=(kc == 0),
                                    stop=(kc == KC - 1),
                                    skip_group_check=True,
                                )
                        for hc in hcs:
                            postproc(eps[hc], hc)
                else:
                    for hc in range(HC):
                        ep = epp.tile([128, 512], f32, tag="ep")
                        last_chunk = blk == NBLK - 1 and hc == HC - 1
                        if last_chunk:
                            # split the very last energy group into two
                            # 256-column halves so the tanh/v-dot chain (and
                            # with it the final partition-sum) starts half a
                            # group earlier - trims the end-of-stream stall
                            tt_l = tanp.tile([128, 512], bf16, tag="tt")
                            tmp_l = tmpp.tile([128, 512], bf16, tag="tmp")
                            for half in range(2):
                                sl = slice(half * 256, half * 256 + 256)
                                for kc in range(KC):
                                    nc.tensor.matmul(
                                        ep[:, sl],
                                        w2sb[:, kc, hc * 128 : (hc + 1) * 128],
                                        etiles[kc][:, sl],
                                        start=(kc == 0),
                                        stop=(kc == KC - 1),
                                        skip_group_check=True,
                                    )
                                nc.scalar.activation(
                                    tt_l[:, sl], ep[:, sl], AF.Tanh,
                                    bias=ub_sb[:, hc, bb : bb + 1], scale=1.0,
                                )
                                nc.vector.tensor_scalar_mul(
                                    tmp_l[:, sl], tt_l[:, sl],
                                    vcol_sb[:, hc : hc + 1],
                                )
                                nc.vector.tensor_tensor(
                                    acc[:, sl], tmp_l[:, sl], acc[:, sl],
                                    op=mybir.AluOpType.add,
                                )
                            continue
                        for kc in range(KC):
                            nc.tensor.matmul(
                                ep[:],
                                w2sb[:, kc, hc * 128 : (hc + 1) * 128],
                                etiles[kc][:],
                                start=(kc == 0),
                                stop=(kc == KC - 1),
                            )
                        if hc == 0 and pending_sum is not None:
                            emit_sum(pending_sum)
                            pending_sum = None
                        postproc(ep, hc)

                pending_sum = (acc, bb, sb)

            emit_sum(pending_sum)

            # partial sums out on the ACT HWDGE queue (parallel with the
            # last esc bank's DMA on the SP queue); host combines + divides.
            # no max-subtraction: |scores| <= ||v||_1 (~25), exp() is safely
            # inside fp32 range, and softmax is shift-invariant
            nc.scalar.dma_start(out_sum[0:B_LOC, :], ssums[0:128:32, :])

    _patch_json(nc)
    return nc


_NC_CACHE = None


def _get_nc():
    global _NC_CACHE
    if _NC_CACHE is None:
        _NC_CACHE = build_kernel()
    return _NC_CACHE


def shard_inputs(hidden, encoderOutputs, W, b, v):
    """Host-side prep: per-core input dict list."""
    import ml_dtypes

    bf16 = ml_dtypes.bfloat16

    hidden = np.ascontiguousarray(hidden, dtype=np.float32)
    W = np.ascontiguousarray(W, dtype=np.float32)
    b = np.ascontiguousarray(b, dtype=np.float32)
    v = np.ascontiguousarray(v, dtype=np.float32)

    w2t = np.ascontiguousarray(W[:, H:].T).astype(bf16)   # [k, h]
    vcol = np.ascontiguousarray(v.reshape(HC, 128).T)     # [128, hc]

    # ubias[b, h] = hidden @ W1^T + b  (tiny [B,H] problem: host fp32)
    ub = hidden @ W[:, :H].T + b[None, :]                 # [B, H]

    # [H, B, S] single big transpose, then per-core contiguous slices
    encT = np.transpose(
        np.asarray(encoderOutputs, dtype=np.float32), (2, 1, 0)
    ).astype(bf16)

    in_maps = []
    for i in range(NCORES):
        b0 = i * B_LOC
        enc_c = np.ascontiguousarray(encT[:, b0 : b0 + B_LOC, :]).reshape(H, R)
        # ubias in [h-part, hc, bb] layout -> [128, HC * B_LOC]
        ub_c = np.ascontiguousarray(
            ub[b0 : b0 + B_LOC].T.reshape(HC, 128, B_LOC).transpose(1, 0, 2)
        ).reshape(128, HC * B_LOC)
        in_maps.append(
            {
                "enc_t": enc_c,
                "w2t": w2t,
                "ubias": ub_c,
                "vcol": vcol,
            }
        )
    return in_maps


def run(in_maps, trace=False):
    if trace:
        _install_ntff_hook()
    from concourse import bass_utils

    nc = _get_nc()
    res = bass_utils.run_bass_kernel_spmd(
        nc, in_maps, core_ids=list(range(NCORES)), trace=trace
    )
    return res


def unshard_output(res):
    """Gather per-core esc/sums and normalize on host."""
    rows = []
    for i in range(NCORES):
        esc = np.asarray(res.results[i]["out_esc"], dtype=np.float64)  # [4, S]
        sums = np.asarray(res.results[i]["out_sum"], dtype=np.float64)  # [4, 4]
        rows.append(esc / sums.sum(axis=1, keepdims=True))
    return np.concatenate(rows, axis=0)[:, None, :].astype(np.float32)


def kernel(hidden, encoderOutputs, W, b, v):
    in_maps = shard_inputs(hidden, encoderOutputs, W, b, v)
    res = run(in_maps, trace=False)
    return unshard_output(res)
